# revision 17
# baseline (speedup 1.0000x reference)
"""Trainium2 Bass kernel for nn_CausalAttention (GNN message passing).

Math (reference):
    pairs[e] = [img[:, src[e]] ; text[:, tgt[e]]]          # B == H == 128
    a[e]     = sigmoid(w2 . relu(W1 @ pairs[e] + b1) + b2) # per-edge gate
    att_img[b, i] = sum_{e: src[e]=i} a[e] * text[b, tgt[e]]
    att_txt[b, t] = sum_{e: tgt[e]=t} a[e] * img[b, src[e]]

v5 architecture: deduplicated edges + host reduction, fp16 on-chip.
Core c owns the edges with src in Wc = [128c, 128c+128). It computes
att_img[:, Wc] exactly plus a PARTIAL att_txt[:, :]; the host sums the
8 partials (no collectives). Per-edge work is done once per edge.

Per core: edges bucketed by w = tgt>>7 (8 buckets, bpw 128-blocks each).
Host ships index-derived one-hot tables (f16):
  ohkt [loc, e] / ohlt [lo, e]  key-major (phase A gathers)
  ohk  [e, loc] / ohlo [e, lo]  edge-major (phase B outer products)
Per bucket w:
  h   = relu(UwinT.T @ ohkt + V8[w].T @ ohlt + b1)   4 matmuls + ACT
  za  = h_blk.T @ w2 (bpw N=1 matmuls), a = sigmoid(za + b2)
  ohka= ohk * a  (ONE broadcast tensor_tensor on DVE)
  M_T[lo, loc] += ohlo_blk.T @ ohka_blk  (bpw matmuls, PSUM group)
  M_N = PE-transpose(M_T) (1 matmul, f16 into bitcast PSUM region)
  acc_img += ttT8[w].T @ M_T             (1 matmul, long PSUM group)
  part[:, wc] = imgwinT.T @ M_N          (1 matmul) -> SBUF -> DMA out
PSUM banks: h x2 (4) + mtp x2 (2: mT | part | transposeT) + acc (1) = 7.
U/V feature transforms built on-chip in fp16 (9 matmuls).
"""

import sys

for _p in ("/opt/trn_rl_repo", "/root/.axon_site/_ro/trn_rl_repo"):
    if _p not in sys.path:
        sys.path.insert(0, _p)

import numpy as np

import concourse.bass as bass
import concourse.tile as tile
from concourse import bacc, mybir

P = 128
DIM = 1024
NCORES = 8
NW = 8            # tgt-hi buckets

F32 = mybir.dt.float32
F16 = mybir.dt.float16

IS_EQ = mybir.AluOpType.is_equal
MULT = mybir.AluOpType.mult
RELU = mybir.ActivationFunctionType.Relu
SIGMOID = mybir.ActivationFunctionType.Sigmoid


def _build_program(bpw):
    nblk = NW * bpw       # blocks total
    bw = bpw * P          # edge slots per bucket
    ec = nblk * P         # edge slots total

    nc = bacc.Bacc(None, target_bir_lowering=False, debug=False)

    txt16_d = nc.dram_tensor("txt16", [P, DIM], F16, kind="ExternalInput")
    ttT8_d = nc.dram_tensor("ttT8", [P, NW * P], F16, kind="ExternalInput")
    iw_d = nc.dram_tensor("iw16", [P, P], F16, kind="ExternalInput")
    iwT_d = nc.dram_tensor("iwT16", [P, P], F16, kind="ExternalInput")
    w1i_d = nc.dram_tensor("w1i16", [P, P], F16, kind="ExternalInput")
    w1x_d = nc.dram_tensor("w1x16", [P, P], F16, kind="ExternalInput")
    cst_d = nc.dram_tensor("cst", [P, 2], F32, kind="ExternalInput")
    w2h_d = nc.dram_tensor("w2h", [P, 1], F16, kind="ExternalInput")
    ohkt_d = nc.dram_tensor("ohkt", [P, ec], F16, kind="ExternalInput")
    ohlt_d = nc.dram_tensor("ohlt", [P, ec], F16, kind="ExternalInput")
    ohlo_d = nc.dram_tensor("ohlo", [P, ec], F16, kind="ExternalInput")
    ohk_d = nc.dram_tensor("ohk", [P, ec], F16, kind="ExternalInput")
    out_img = nc.dram_tensor("out_img", [P, P], F32, kind="ExternalOutput")
    out_part = nc.dram_tensor("out_part", [P, DIM], F32, kind="ExternalOutput")

    HW = 640  # h psum cols; za tail lives at cols HW:HW+bpw (same 2nd bank)

    with tile.TileContext(nc) as tc:
        with (
            tc.tile_pool(name="const", bufs=1) as cp,
            tc.tile_pool(name="work", bufs=3) as wp,
            tc.tile_pool(name="psh", bufs=2, space="PSUM") as psh,
            tc.tile_pool(name="psm", bufs=2, space="PSUM") as psm,
            tc.tile_pool(name="pso", bufs=1, space="PSUM") as pso,
        ):
            txt16 = cp.tile([P, DIM], F16)
            ttT8 = cp.tile([P, NW, P], F16)
            iw_s = cp.tile([P, P], F16)
            iwT_s = cp.tile([P, P], F16)
            w1i_s = cp.tile([P, P], F16)
            w1x_s = cp.tile([P, P], F16)
            cst_s = cp.tile([P, 2], F32)
            w2h_s = cp.tile([P, 1], F16)
            ohkt_s = cp.tile([P, ec], F16)
            ohlt_s = cp.tile([P, ec], F16)
            ohlo_s = cp.tile([P, ec], F16)
            ohk_s = cp.tile([P, ec], F16)
            iota16 = cp.tile([P, P], F16)
            iota_i = cp.tile([P, 1], mybir.dt.int32)
            iota_cf = cp.tile([P, 1], F32)
            ident16 = cp.tile([P, P], F16)
            V8 = cp.tile([P, NW, P], F16)
            UwinT = cp.tile([P, P], F16)
            a_s = cp.tile([P, nblk], F32)

            # small loads first (builds need them), then per-bucket chunks
            nc.scalar.dma_start(cst_s[:], cst_d[:])
            nc.scalar.dma_start(w2h_s[:], w2h_d[:])
            nc.scalar.dma_start(w1i_s[:], w1i_d[:])
            nc.scalar.dma_start(w1x_s[:], w1x_d[:])
            nc.scalar.dma_start(iw_s[:], iw_d[:])
            nc.scalar.dma_start(iwT_s[:], iwT_d[:])
            nc.scalar.dma_start(txt16[:], txt16_d[:])
            nc.scalar.dma_start(
                ttT8[:], ttT8_d[:].rearrange("p (w b) -> p w b", w=NW)
            )
            for w in range(NW):
                sl = slice(w * bw, (w + 1) * bw)
                nc.sync.dma_start(ohkt_s[:, sl], ohkt_d[:, sl])
                nc.scalar.dma_start(ohlt_s[:, sl], ohlt_d[:, sl])
                nc.sync.dma_start(ohlo_s[:, sl], ohlo_d[:, sl])
                nc.scalar.dma_start(ohk_s[:, sl], ohk_d[:, sl])
            b1c = cst_s[:, 0:1]
            b2c = cst_s[:, 1:2]

            nc.gpsimd.iota(
                iota16[:], pattern=[[1, P]], base=0, channel_multiplier=0,
                allow_small_or_imprecise_dtypes=True,
            )
            nc.gpsimd.iota(iota_i[:], pattern=[[0, 1]], base=0,
                           channel_multiplier=1)
            nc.vector.tensor_copy(iota_cf[:], iota_i[:])
            nc.vector.tensor_scalar(
                out=ident16[:], in0=iota16[:], scalar1=iota_cf[:, 0:1],
                scalar2=None, op0=IS_EQ,
            )

            # on-chip feature transforms: UwinT[loc,h], V8[lo,w,h] (fp16)
            for k, (lhs, rhs, dst) in enumerate(
                [(iw_s[:], w1i_s[:], UwinT[:])]
                + [
                    (txt16[:, w * P : (w + 1) * P], w1x_s[:], V8[:, w, :])
                    for w in range(NW)
                ]
            ):
                bp = psh.tile([P, HW + 8], F32, tag="h", name=f"bld{k}")
                nc.tensor.matmul(bp[:, 0:P], lhs, rhs, start=True, stop=True)
                nc.scalar.copy(dst, bp[:, 0:P])

            acc = pso.tile([P, P], F32, tag="acc")
            for w in range(NW):
                e0 = w * bw
                # ---- phase A: h = relu(U-term + V-term + b1) ----
                h_ps = psh.tile([P, HW + 8], F32, tag="h")
                for o, n in ((0, 512), (512, bw - 512)):
                    nc.tensor.matmul(
                        h_ps[:, o : o + n], UwinT[:],
                        ohkt_s[:, e0 + o : e0 + o + n],
                        start=True, stop=False,
                    )
                    nc.tensor.matmul(
                        h_ps[:, o : o + n], V8[:, w, :],
                        ohlt_s[:, e0 + o : e0 + o + n],
                        start=False, stop=True,
                    )
                h16 = wp.tile([P, bw], F16, tag="h16")
                nc.scalar.activation(h16[:], h_ps[:, 0:bw], RELU, bias=b1c)
                # ---- za[e] = h_blk.T @ w2; a = sigmoid(za + b2) ----
                for j in range(bpw):
                    nc.tensor.matmul(
                        h_ps[:, HW + j : HW + j + 1],
                        h16[:, j * P : (j + 1) * P], w2h_s[:],
                        start=True, stop=True, skip_group_check=True,
                    )
                nc.scalar.activation(
                    a_s[:, w * bpw : (w + 1) * bpw],
                    h_ps[:, HW : HW + bpw], SIGMOID, bias=b2c,
                )
                # ---- phase B: ohka = ohk * a (one broadcast mult) ----
                ohkaB = wp.tile([P, bw], F16, tag="ohka")
                a_bc = a_s[:, w * bpw : (w + 1) * bpw, None].broadcast_to(
                    (P, bpw, P)
                )
                nc.vector.tensor_tensor(
                    out=ohkaB[:], in0=ohk_s[:, e0 : e0 + bw], in1=a_bc,
                    op=MULT,
                )
                # mtp bank layout (f32 cols): [0:128] M_T accum,
                # [128:256] part chunk, [256:320] M_N (f16 via bitcast)
                mtp = psm.tile([P, 384], F32, tag="mtp")
                for j in range(bpw):
                    sl = slice(e0 + j * P, e0 + (j + 1) * P)
                    nc.tensor.matmul(
                        mtp[:, 0:P], ohlo_s[:, sl], ohkaB[:, j * P : (j + 1) * P],
                        start=(j == 0), stop=(j == bpw - 1),
                        skip_group_check=True,
                    )
                m16T = wp.tile([P, P], F16, tag="m16T")
                nc.vector.tensor_copy(m16T[:], mtp[:, 0:P])
                mN_ps = mtp[:, 2 * P : 2 * P + P // 2].bitcast(F16)
                nc.tensor.matmul(
                    mN_ps, m16T[:], ident16[:], is_transpose=True,
                    start=True, stop=True, skip_group_check=True,
                )
                m16N = wp.tile([P, P], F16, tag="m16N")
                nc.scalar.copy(m16N[:], mN_ps)
                # ---- tails ----
                nc.tensor.matmul(
                    acc[:], ttT8[:, w, :], m16T[:],
                    start=(w == 0), stop=(w == NW - 1), skip_group_check=True,
                )
                nc.tensor.matmul(
                    mtp[:, P : 2 * P], iwT_s[:], m16N[:],
                    start=True, stop=True, skip_group_check=True,
                )
                part_sb = wp.tile([P, P], F32, tag="part_sb")
                nc.vector.tensor_copy(part_sb[:], mtp[:, P : 2 * P])
                nc.sync.dma_start(out_part[:, w * P : (w + 1) * P], part_sb[:])

            out_sb = wp.tile([P, P], F32, tag="out_sb")
            nc.scalar.copy(out_sb[:], acc[:])
            nc.sync.dma_start(out_img[:], out_sb[:])

    nc.compile()
    return nc


_PROGRAMS = {}


def _get_program(bpw):
    if bpw not in _PROGRAMS:
        _PROGRAMS[bpw] = _build_program(bpw)
    return _PROGRAMS[bpw]


def _core_arrays(kloc, arb, bpw):
    """kloc: src-base (0..127) for this core's edges; arb: tgt values.
    Returns ohkt, ohlt [P, ec] (key-major) and ohk, ohlo [P, ec]
    (edge-major, per-block [e, key] tiles), all f16."""
    nblk = NW * bpw
    bw = bpw * P
    ec = nblk * P
    w = arb >> 7
    lo = arb & 127
    klocs = np.full(ec, -1, np.int64)
    los = np.full(ec, -1, np.int64)
    fill = np.zeros(NW, np.int64)
    order = np.argsort(w, kind="stable")
    for ei in order:
        wb = w[ei]
        s = wb * bw + fill[wb]
        klocs[s] = kloc[ei]
        los[s] = lo[ei]
        fill[wb] += 1
    rng = np.arange(P)
    ohkt = np.ascontiguousarray((klocs[None, :] == rng[:, None]).astype(np.float16))
    ohlt = np.ascontiguousarray((los[None, :] == rng[:, None]).astype(np.float16))
    # edge-major: oh*[e % P, b*P + key] = (key_e == key)
    ohlo = np.zeros((P, ec), np.float16)
    ohk = np.zeros((P, ec), np.float16)
    losb = los.reshape(nblk, P)
    klocsb = klocs.reshape(nblk, P)
    for b in range(nblk):
        ohlo[:, b * P : (b + 1) * P] = losb[b][:, None] == rng[None, :]
        ohk[:, b * P : (b + 1) * P] = klocsb[b][:, None] == rng[None, :]
    return ohkt, ohlt, np.ascontiguousarray(ohk), np.ascontiguousarray(ohlo)


def _make_in_maps(img_features, text_features, src, tgt, W1, b1, w2, b2, bpw):
    img = np.asarray(img_features, dtype=np.float32)
    txt = np.asarray(text_features, dtype=np.float32)
    src = np.asarray(src).astype(np.int64)
    tgt = np.asarray(tgt).astype(np.int64)
    txt16 = np.ascontiguousarray(txt.astype(np.float16))
    txtT = txt.T.astype(np.float16)                     # [1024, 128]
    ttT8 = np.ascontiguousarray(
        txtT.reshape(NW, P, P).transpose(1, 0, 2).reshape(P, NW * P)
    )                                                   # [lo, w*128+b]
    w1i16 = np.ascontiguousarray(W1[:, :P].T.astype(np.float16))
    w1x16 = np.ascontiguousarray(W1[:, P:].T.astype(np.float16))
    cst = np.ascontiguousarray(
        np.stack(
            [np.asarray(b1, np.float32),
             np.full(P, np.float32(b2), np.float32)], axis=1)
    )
    w2h = np.ascontiguousarray(np.asarray(w2, np.float16).reshape(P, 1))

    in_maps = []
    for c in range(NCORES):
        base = c * P
        sel = (src >= base) & (src < base + P)
        ohkt, ohlt, ohk, ohlo = _core_arrays(src[sel] - base, tgt[sel], bpw)
        iw = img[:, base : base + P].astype(np.float16)
        m = {
            "txt16": txt16, "ttT8": ttT8,
            "iw16": np.ascontiguousarray(iw),
            "iwT16": np.ascontiguousarray(iw.T),
            "w1i16": w1i16, "w1x16": w1x16,
            "cst": cst, "w2h": w2h,
            "ohkt": ohkt, "ohlt": ohlt, "ohlo": ohlo, "ohk": ohk,
        }
        in_maps.append(m)
    return in_maps


def _pick_bpw(src, tgt):
    src = np.asarray(src).astype(np.int64)
    tgt = np.asarray(tgt).astype(np.int64)
    mx = 0
    for c in range(NCORES):
        sel = (src >> 7) == c
        w = tgt[sel] >> 7
        mx = max(mx, int(np.bincount(w, minlength=NW).max()))
    return (mx + P - 1) // P


def _run(inputs, trace=False):
    from concourse.bass_utils import run_bass_kernel_spmd

    bpw = _pick_bpw(inputs["src"], inputs["tgt"])
    nc = _get_program(bpw)
    in_maps = _make_in_maps(**inputs, bpw=bpw)
    res = run_bass_kernel_spmd(
        nc, in_maps, core_ids=list(range(NCORES)), trace=trace
    )
    att_img = np.concatenate([r["out_img"] for r in res.results], axis=1)
    att_txt = np.sum([r["out_part"] for r in res.results], axis=0)
    return (
        np.ascontiguousarray(att_img.astype(np.float32)),
        np.ascontiguousarray(att_txt.astype(np.float32)),
    ), res


def kernel(**inputs):
    out, _ = _run(inputs, trace=False)
    return out


# revision 24
# speedup vs baseline: 1.3013x; 1.3013x over previous
"""Trainium2 Bass kernel for nn_CausalAttention (GNN message passing).

Math (reference):
    pairs[e] = [img[:, src[e]] ; text[:, tgt[e]]]          # B == H == 128
    a[e]     = sigmoid(w2 . relu(W1 @ pairs[e] + b1) + b2) # per-edge gate
    att_img[b, i] = sum_{e: src[e]=i} a[e] * text[b, tgt[e]]
    att_txt[b, t] = sum_{e: tgt[e]=t} a[e] * img[b, src[e]]

v5 architecture: deduplicated edges + host reduction, fp16 on-chip.
Core c owns the edges with src in Wc = [128c, 128c+128). It computes
att_img[:, Wc] exactly plus a PARTIAL att_txt[:, :]; the host sums the
8 partials (no collectives). Per-edge work is done once per edge.

Per core: edges bucketed by w = tgt>>7 (8 buckets, bpw 128-blocks each).
Host ships index-derived one-hot tables (f16):
  ohkt [loc, e] / ohlt [lo, e]  key-major (phase A gathers)
  ohk  [e, loc] / ohlo [e, lo]  edge-major (phase B outer products)
Per bucket w:
  h   = relu(UwinT.T @ ohkt + V8[w].T @ ohlt + b1)   4 matmuls + ACT
  za  = h_blk.T @ w2 (bpw N=1 matmuls), a = sigmoid(za + b2)
  ohka= ohk * a  (ONE broadcast tensor_tensor on DVE)
  M_T[lo, loc] += ohlo_blk.T @ ohka_blk  (bpw matmuls, PSUM group)
  M_N = PE-transpose(M_T) (1 matmul, f16 into bitcast PSUM region)
  acc_img += ttT8[w].T @ M_T             (1 matmul, long PSUM group)
  part[:, wc] = imgwinT.T @ M_N          (1 matmul) -> SBUF -> DMA out
PSUM banks: h x2 (4) + mtp x2 (2: mT | part | transposeT) + acc (1) = 7.
U/V feature transforms built on-chip in fp16 (9 matmuls).
"""

import sys

for _p in ("/opt/trn_rl_repo", "/root/.axon_site/_ro/trn_rl_repo"):
    if _p not in sys.path:
        sys.path.insert(0, _p)

import numpy as np

import concourse.bass as bass
import concourse.tile as tile
from concourse import bacc, mybir

P = 128
DIM = 1024
NCORES = 8
NW = 8            # tgt-hi buckets

F32 = mybir.dt.float32
F16 = mybir.dt.float16
F8 = mybir.dt.float8e4

IS_EQ = mybir.AluOpType.is_equal
MULT = mybir.AluOpType.mult
RELU = mybir.ActivationFunctionType.Relu
SIGMOID = mybir.ActivationFunctionType.Sigmoid


def _build_program(bpw):
    nblk = NW * bpw       # blocks total
    bw = bpw * P          # edge slots per bucket
    ec = nblk * P         # edge slots total

    nc = bacc.Bacc(None, target_bir_lowering=False, debug=False)

    txt16_d = nc.dram_tensor("txt16", [P, DIM], F16, kind="ExternalInput")
    ttT8_d = nc.dram_tensor("ttT8", [P, NW * P], F16, kind="ExternalInput")
    iw_d = nc.dram_tensor("iw16", [P, P], F16, kind="ExternalInput")
    iwT_d = nc.dram_tensor("iwT16", [P, P], F16, kind="ExternalInput")
    w1i_d = nc.dram_tensor("w1i16", [P, P], F16, kind="ExternalInput")
    w1x_d = nc.dram_tensor("w1x16", [P, P], F16, kind="ExternalInput")
    cst_d = nc.dram_tensor("cst", [P, 2], F32, kind="ExternalInput")
    w2h_d = nc.dram_tensor("w2h", [P, 1], F16, kind="ExternalInput")
    ohkt_d = nc.dram_tensor("ohkt", [P, ec], F8, kind="ExternalInput")
    ohlt_d = nc.dram_tensor("ohlt", [P, ec], F8, kind="ExternalInput")
    ohlo_d = nc.dram_tensor("ohlo", [P, ec], F8, kind="ExternalInput")
    ohk_d = nc.dram_tensor("ohk", [P, ec], F8, kind="ExternalInput")
    out_img = nc.dram_tensor("out_img", [P, P], F32, kind="ExternalOutput")
    out_part = nc.dram_tensor("out_part", [P, DIM], F32, kind="ExternalOutput")

    HW = 640  # h psum cols; za tail lives at cols HW:HW+bpw (same 2nd bank)

    with tile.TileContext(nc) as tc:
        with (
            tc.tile_pool(name="const", bufs=1) as cp,
            tc.tile_pool(name="work", bufs=3) as wp,
            tc.tile_pool(name="psh", bufs=2, space="PSUM") as psh,
            tc.tile_pool(name="psm", bufs=2, space="PSUM") as psm,
            tc.tile_pool(name="pso", bufs=1, space="PSUM") as pso,
        ):
            txt16 = cp.tile([P, DIM], F16)
            ttT8 = cp.tile([P, NW, P], F16)
            iw_s = cp.tile([P, P], F16)
            iwT_s = cp.tile([P, P], F16)
            w1i_s = cp.tile([P, P], F16)
            w1x_s = cp.tile([P, P], F16)
            cst_s = cp.tile([P, 2], F32)
            w2h_s = cp.tile([P, 1], F16)
            ohkt_s = cp.tile([P, ec], F8)
            ohlt_s = cp.tile([P, ec], F8)
            ohlo_s = cp.tile([P, ec], F8)
            ohk_s = cp.tile([P, ec], F8)
            part_all = cp.tile([P, DIM], F32)
            iota16 = cp.tile([P, P], F16)
            iota_i = cp.tile([P, 1], mybir.dt.int32)
            iota_cf = cp.tile([P, 1], F32)
            ident16 = cp.tile([P, P], F16)
            V8 = cp.tile([P, NW, P], F16)
            UwinT = cp.tile([P, P], F16)
            a_s = cp.tile([P, nblk], F32)

            # DMA plan: scalar(Act) queue issues only the 4 big chunks of
            # ohlt/ohk; sync(SP) queue gets builds' inputs first, then
            # ohkt/ohlo halves, then the rest. Tables are fp8 (hold 0/1).
            HC = ec // 2
            nc.scalar.dma_start(ohlt_s[:, 0:HC], ohlt_d[:, 0:HC])
            nc.scalar.dma_start(ohk_s[:, 0:HC], ohk_d[:, 0:HC])
            nc.scalar.dma_start(ohlt_s[:, HC:ec], ohlt_d[:, HC:ec])
            nc.scalar.dma_start(ohk_s[:, HC:ec], ohk_d[:, HC:ec])
            nc.sync.dma_start(txt16[:], txt16_d[:])
            nc.sync.dma_start(w1x_s[:], w1x_d[:])
            nc.sync.dma_start(iw_s[:], iw_d[:])
            nc.sync.dma_start(w1i_s[:], w1i_d[:])
            nc.sync.dma_start(ohkt_s[:, 0:HC], ohkt_d[:, 0:HC])
            nc.sync.dma_start(cst_s[:], cst_d[:])
            nc.sync.dma_start(w2h_s[:], w2h_d[:])
            nc.sync.dma_start(iwT_s[:], iwT_d[:])
            nc.sync.dma_start(
                ttT8[:], ttT8_d[:].rearrange("p (w b) -> p w b", w=NW)
            )
            nc.sync.dma_start(ohlo_s[:, 0:HC], ohlo_d[:, 0:HC])
            nc.sync.dma_start(ohkt_s[:, HC:ec], ohkt_d[:, HC:ec])
            nc.sync.dma_start(ohlo_s[:, HC:ec], ohlo_d[:, HC:ec])
            b1c = cst_s[:, 0:1]
            b2c = cst_s[:, 1:2]

            nc.gpsimd.iota(
                iota16[:], pattern=[[1, P]], base=0, channel_multiplier=0,
                allow_small_or_imprecise_dtypes=True,
            )
            nc.gpsimd.iota(iota_i[:], pattern=[[0, 1]], base=0,
                           channel_multiplier=1)
            nc.vector.tensor_copy(iota_cf[:], iota_i[:])
            nc.vector.tensor_scalar(
                out=ident16[:], in0=iota16[:], scalar1=iota_cf[:, 0:1],
                scalar2=None, op0=IS_EQ,
            )

            # on-chip feature transforms: UwinT[loc,h], V8[lo,w,h] (fp16)
            for k, (lhs, rhs, dst) in enumerate(
                [(iw_s[:], w1i_s[:], UwinT[:])]
                + [
                    (txt16[:, w * P : (w + 1) * P], w1x_s[:], V8[:, w, :])
                    for w in range(NW)
                ]
            ):
                bp = psh.tile([P, HW + 8], F32, tag="h", name=f"bld{k}")
                nc.tensor.matmul(bp[:, 0:P], lhs, rhs, start=True, stop=True)
                nc.scalar.copy(dst, bp[:, 0:P])

            acc = pso.tile([P, P], F32, tag="acc")
            for w in range(NW):
                e0 = w * bw
                # ---- phase A: h = relu(U-term + V-term + b1) ----
                h_ps = psh.tile([P, HW + 8], F32, tag="h")
                for o, n in ((0, 512), (512, bw - 512)):
                    nc.tensor.matmul(
                        h_ps[:, o : o + n], UwinT[:],
                        ohkt_s[:, e0 + o : e0 + o + n],
                        start=True, stop=False,
                    )
                    nc.tensor.matmul(
                        h_ps[:, o : o + n], V8[:, w, :],
                        ohlt_s[:, e0 + o : e0 + o + n],
                        start=False, stop=True,
                    )
                h16 = wp.tile([P, bw], F16, tag="h16")
                nc.scalar.activation(h16[:], h_ps[:, 0:bw], RELU, bias=b1c)
                # ---- za[e] = h_blk.T @ w2; a = sigmoid(za + b2) ----
                for j in range(bpw):
                    nc.tensor.matmul(
                        h_ps[:, HW + j : HW + j + 1],
                        h16[:, j * P : (j + 1) * P], w2h_s[:],
                        start=True, stop=True, skip_group_check=True,
                    )
                nc.scalar.activation(
                    a_s[:, w * bpw : (w + 1) * bpw],
                    h_ps[:, HW : HW + bpw], SIGMOID, bias=b2c,
                )
                # ---- phase B: ohka = ohk * a (one broadcast mult) ----
                ohkaB = wp.tile([P, bw], F16, tag="ohka")
                a_bc = a_s[:, w * bpw : (w + 1) * bpw, None].broadcast_to(
                    (P, bpw, P)
                )
                nc.vector.tensor_tensor(
                    out=ohkaB[:], in0=ohk_s[:, e0 : e0 + bw], in1=a_bc,
                    op=MULT,
                )
                # mtp bank layout (f32 cols): [0:128] M_T accum,
                # [128:256] part chunk, [256:320] M_N (f16 via bitcast)
                mtp = psm.tile([P, 384], F32, tag="mtp")
                for j in range(bpw):
                    sl = slice(e0 + j * P, e0 + (j + 1) * P)
                    nc.tensor.matmul(
                        mtp[:, 0:P], ohlo_s[:, sl], ohkaB[:, j * P : (j + 1) * P],
                        start=(j == 0), stop=(j == bpw - 1),
                        skip_group_check=True,
                    )
                m16T = wp.tile([P, P], F16, tag="m16T")
                nc.vector.tensor_copy(m16T[:], mtp[:, 0:P])
                mN_ps = mtp[:, 2 * P : 2 * P + P // 2].bitcast(F16)
                nc.tensor.matmul(
                    mN_ps, m16T[:], ident16[:], is_transpose=True,
                    start=True, stop=True, skip_group_check=True,
                )
                m16N = wp.tile([P, P], F16, tag="m16N")
                nc.vector.tensor_copy(m16N[:], mN_ps)
                # ---- tails ----
                nc.tensor.matmul(
                    acc[:], ttT8[:, w, :], m16T[:],
                    start=(w == 0), stop=(w == NW - 1), skip_group_check=True,
                )
                nc.tensor.matmul(
                    mtp[:, P : 2 * P], iwT_s[:], m16N[:],
                    start=True, stop=True, skip_group_check=True,
                )
                nc.vector.tensor_copy(
                    part_all[:, w * P : (w + 1) * P], mtp[:, P : 2 * P]
                )
                if w == NW // 2 - 1:
                    nc.sync.dma_start(
                        out_part[:, 0 : DIM // 2], part_all[:, 0 : DIM // 2]
                    )
                elif w == NW - 1:
                    nc.sync.dma_start(
                        out_part[:, DIM // 2 : DIM], part_all[:, DIM // 2 : DIM]
                    )

            out_sb = wp.tile([P, P], F32, tag="out_sb")
            nc.scalar.copy(out_sb[:], acc[:])
            nc.sync.dma_start(out_img[:], out_sb[:])

    nc.compile()
    return nc


_PROGRAMS = {}


def _get_program(bpw):
    if bpw not in _PROGRAMS:
        _PROGRAMS[bpw] = _build_program(bpw)
    return _PROGRAMS[bpw]


def _core_arrays(kloc, arb, bpw):
    """kloc: src-base (0..127) for this core's edges; arb: tgt values.
    Returns ohkt, ohlt [P, ec] (key-major) and ohk, ohlo [P, ec]
    (edge-major, per-block [e, key] tiles), all f16."""
    nblk = NW * bpw
    bw = bpw * P
    ec = nblk * P
    w = arb >> 7
    lo = arb & 127
    klocs = np.full(ec, -1, np.int64)
    los = np.full(ec, -1, np.int64)
    fill = np.zeros(NW, np.int64)
    order = np.argsort(w, kind="stable")
    for ei in order:
        wb = w[ei]
        s = wb * bw + fill[wb]
        klocs[s] = kloc[ei]
        los[s] = lo[ei]
        fill[wb] += 1
    import ml_dtypes

    f8 = ml_dtypes.float8_e4m3
    rng = np.arange(P)
    ohkt = np.ascontiguousarray((klocs[None, :] == rng[:, None]).astype(f8))
    ohlt = np.ascontiguousarray((los[None, :] == rng[:, None]).astype(f8))
    # edge-major: oh*[e % P, b*P + key] = (key_e == key)
    ohlo = np.zeros((P, ec), f8)
    ohk = np.zeros((P, ec), f8)
    losb = los.reshape(nblk, P)
    klocsb = klocs.reshape(nblk, P)
    for b in range(nblk):
        ohlo[:, b * P : (b + 1) * P] = (losb[b][:, None] == rng[None, :]).astype(f8)
        ohk[:, b * P : (b + 1) * P] = (klocsb[b][:, None] == rng[None, :]).astype(f8)
    return ohkt, ohlt, np.ascontiguousarray(ohk), np.ascontiguousarray(ohlo)


def _make_in_maps(img_features, text_features, src, tgt, W1, b1, w2, b2, bpw):
    img = np.asarray(img_features, dtype=np.float32)
    txt = np.asarray(text_features, dtype=np.float32)
    src = np.asarray(src).astype(np.int64)
    tgt = np.asarray(tgt).astype(np.int64)
    txt16 = np.ascontiguousarray(txt.astype(np.float16))
    txtT = txt.T.astype(np.float16)                     # [1024, 128]
    ttT8 = np.ascontiguousarray(
        txtT.reshape(NW, P, P).transpose(1, 0, 2).reshape(P, NW * P)
    )                                                   # [lo, w*128+b]
    w1i16 = np.ascontiguousarray(W1[:, :P].T.astype(np.float16))
    w1x16 = np.ascontiguousarray(W1[:, P:].T.astype(np.float16))
    cst = np.ascontiguousarray(
        np.stack(
            [np.asarray(b1, np.float32),
             np.full(P, np.float32(b2), np.float32)], axis=1)
    )
    w2h = np.ascontiguousarray(np.asarray(w2, np.float16).reshape(P, 1))

    in_maps = []
    for c in range(NCORES):
        base = c * P
        sel = (src >= base) & (src < base + P)
        ohkt, ohlt, ohk, ohlo = _core_arrays(src[sel] - base, tgt[sel], bpw)
        iw = img[:, base : base + P].astype(np.float16)
        m = {
            "txt16": txt16, "ttT8": ttT8,
            "iw16": np.ascontiguousarray(iw),
            "iwT16": np.ascontiguousarray(iw.T),
            "w1i16": w1i16, "w1x16": w1x16,
            "cst": cst, "w2h": w2h,
            "ohkt": ohkt, "ohlt": ohlt, "ohlo": ohlo, "ohk": ohk,
        }
        in_maps.append(m)
    return in_maps


def _pick_bpw(src, tgt):
    src = np.asarray(src).astype(np.int64)
    tgt = np.asarray(tgt).astype(np.int64)
    mx = 0
    for c in range(NCORES):
        sel = (src >> 7) == c
        w = tgt[sel] >> 7
        mx = max(mx, int(np.bincount(w, minlength=NW).max()))
    return (mx + P - 1) // P


def _run(inputs, trace=False):
    from concourse.bass_utils import run_bass_kernel_spmd

    bpw = _pick_bpw(inputs["src"], inputs["tgt"])
    nc = _get_program(bpw)
    in_maps = _make_in_maps(**inputs, bpw=bpw)
    res = run_bass_kernel_spmd(
        nc, in_maps, core_ids=list(range(NCORES)), trace=trace
    )
    att_img = np.concatenate([r["out_img"] for r in res.results], axis=1)
    att_txt = np.sum([r["out_part"] for r in res.results], axis=0)
    return (
        np.ascontiguousarray(att_img.astype(np.float32)),
        np.ascontiguousarray(att_txt.astype(np.float32)),
    ), res


def kernel(**inputs):
    out, _ = _run(inputs, trace=False)
    return out


# revision 30
# speedup vs baseline: 1.4336x; 1.1016x over previous
"""Trainium2 Bass kernel for nn_CausalAttention (GNN message passing).

Math (reference):
    pairs[e] = [img[:, src[e]] ; text[:, tgt[e]]]          # B == H == 128
    a[e]     = sigmoid(w2 . relu(W1 @ pairs[e] + b1) + b2) # per-edge gate
    att_img[b, i] = sum_{e: src[e]=i} a[e] * text[b, tgt[e]]
    att_txt[b, t] = sum_{e: tgt[e]=t} a[e] * img[b, src[e]]

v5 architecture: deduplicated edges + host reduction, fp16 on-chip.
Core c owns the edges with src in Wc = [128c, 128c+128). It computes
att_img[:, Wc] exactly plus a PARTIAL att_txt[:, :]; the host sums the
8 partials (no collectives). Per-edge work is done once per edge.

Per core: edges bucketed by w = tgt>>7 (8 buckets, bpw 128-blocks each).
Host ships index-derived one-hot tables (f16):
  ohkt [loc, e] / ohlt [lo, e]  key-major (phase A gathers)
  ohk  [e, loc] / ohlo [e, lo]  edge-major (phase B outer products)
Per bucket w:
  h   = relu(UwinT.T @ ohkt + V8[w].T @ ohlt + b1)   4 matmuls + ACT
  za  = h_blk.T @ w2 (bpw N=1 matmuls), a = sigmoid(za + b2)
  ohka= ohk * a  (ONE broadcast tensor_tensor on DVE)
  M_T[lo, loc] += ohlo_blk.T @ ohka_blk  (bpw matmuls, PSUM group)
  M_N = PE-transpose(M_T) (1 matmul, f16 into bitcast PSUM region)
  acc_img += ttT8[w].T @ M_T             (1 matmul, long PSUM group)
  part[:, wc] = imgwinT.T @ M_N          (1 matmul) -> SBUF -> DMA out
PSUM banks: h x2 (4) + mtp x2 (2: mT | part | transposeT) + acc (1) = 7.
U/V feature transforms built on-chip in fp16 (9 matmuls).
"""

import sys

for _p in ("/opt/trn_rl_repo", "/root/.axon_site/_ro/trn_rl_repo"):
    if _p not in sys.path:
        sys.path.insert(0, _p)

import numpy as np

import concourse.bass as bass
import concourse.tile as tile
from concourse import bacc, mybir

P = 128
DIM = 1024
NCORES = 8
NW = 8            # tgt-hi buckets

F32 = mybir.dt.float32
F16 = mybir.dt.float16
F8 = mybir.dt.float8e4

IS_EQ = mybir.AluOpType.is_equal
MULT = mybir.AluOpType.mult
RELU = mybir.ActivationFunctionType.Relu
SIGMOID = mybir.ActivationFunctionType.Sigmoid


def _build_program(bpw):
    nblk = NW * bpw       # blocks total
    bw = bpw * P          # edge slots per bucket
    ec = nblk * P         # edge slots total

    nc = bacc.Bacc(None, target_bir_lowering=False, debug=False)

    txt16_d = nc.dram_tensor("txt16", [P, DIM], F16, kind="ExternalInput")
    ttT8_d = nc.dram_tensor("ttT8", [P, NW * P], F16, kind="ExternalInput")
    # blob: w1i | w1x | iw | iwT | w2h  (f16, 4*128+1 cols)
    blob_d = nc.dram_tensor("blob16", [P, 4 * P + 1], F16, kind="ExternalInput")
    cst_d = nc.dram_tensor("cst", [P, 2], F32, kind="ExternalInput")
    ohkt_d = nc.dram_tensor("ohkt", [P, ec], F8, kind="ExternalInput")
    ohlt_d = nc.dram_tensor("ohlt", [P, ec], F8, kind="ExternalInput")
    ohlo_d = nc.dram_tensor("ohlo", [P, ec], F8, kind="ExternalInput")
    ohk_d = nc.dram_tensor("ohk", [P, ec], F8, kind="ExternalInput")
    out_img = nc.dram_tensor("out_img", [P, P], F32, kind="ExternalOutput")
    out_part = nc.dram_tensor("out_part", [P, DIM], F32, kind="ExternalOutput")

    HW = 640  # h psum cols; za tail lives at cols HW:HW+bpw (same 2nd bank)

    with tile.TileContext(nc) as tc:
        with (
            tc.tile_pool(name="const", bufs=1) as cp,
            tc.tile_pool(name="work", bufs=4) as wp,
            tc.tile_pool(name="psh", bufs=2, space="PSUM") as psh,
            tc.tile_pool(name="psm", bufs=3, space="PSUM") as psm,
            tc.tile_pool(name="pso", bufs=1, space="PSUM") as pso,
        ):
            txt16 = cp.tile([P, DIM], F16)
            ttT8 = cp.tile([P, NW, P], F16)
            blob_s = cp.tile([P, 4 * P + 1], F16)
            cst_s = cp.tile([P, 2], F32)
            w1i_s = blob_s[:, 0:P]
            w1x_s = blob_s[:, P : 2 * P]
            iw_s = blob_s[:, 2 * P : 3 * P]
            iwT_s = blob_s[:, 3 * P : 4 * P]
            w2h_s = blob_s[:, 4 * P : 4 * P + 1]
            ohkt_s = cp.tile([P, ec], F8)
            ohlt_s = cp.tile([P, ec], F8)
            ohlo_s = cp.tile([P, ec], F8)
            ohk_s = cp.tile([P, ec], F8)
            part_all = cp.tile([P, DIM], F32)
            iota16 = cp.tile([P, P], F16)
            iota_i = cp.tile([P, 1], mybir.dt.int32)
            iota_cf = cp.tile([P, 1], F32)
            ident16 = cp.tile([P, P], F16)
            V8 = cp.tile([P, NW, P], F16)
            UwinT = cp.tile([P, P], F16)
            a_s = cp.tile([P, nblk], F32)

            # DMA plan: scalar(Act) queue issues only the big chunks of
            # ohlt/ohk; sync(SP) queue gets builds' inputs first, then
            # ohkt/ohlo chunks. Tables are fp8 (hold only 0/1). First
            # chunk covers buckets 0-1 so compute starts early.
            C0 = 2 * bw
            nc.scalar.dma_start(ohlt_s[:, 0:C0], ohlt_d[:, 0:C0])
            nc.scalar.dma_start(ohk_s[:, 0:C0], ohk_d[:, 0:C0])
            nc.scalar.dma_start(ohlt_s[:, C0:ec], ohlt_d[:, C0:ec])
            nc.scalar.dma_start(ohk_s[:, C0:ec], ohk_d[:, C0:ec])
            nc.sync.dma_start(txt16[:], txt16_d[:])
            nc.sync.dma_start(blob_s[:], blob_d[:])
            nc.sync.dma_start(cst_s[:], cst_d[:])
            nc.sync.dma_start(ohkt_s[:, 0:C0], ohkt_d[:, 0:C0])
            nc.sync.dma_start(ohlo_s[:, 0:C0], ohlo_d[:, 0:C0])
            nc.sync.dma_start(
                ttT8[:], ttT8_d[:].rearrange("p (w b) -> p w b", w=NW)
            )
            nc.sync.dma_start(ohkt_s[:, C0:ec], ohkt_d[:, C0:ec])
            nc.sync.dma_start(ohlo_s[:, C0:ec], ohlo_d[:, C0:ec])
            b1c = cst_s[:, 0:1]
            b2c = cst_s[:, 1:2]

            nc.gpsimd.iota(
                iota16[:], pattern=[[1, P]], base=0, channel_multiplier=0,
                allow_small_or_imprecise_dtypes=True,
            )
            nc.gpsimd.iota(iota_i[:], pattern=[[0, 1]], base=0,
                           channel_multiplier=1)
            nc.vector.tensor_copy(iota_cf[:], iota_i[:])
            nc.vector.tensor_scalar(
                out=ident16[:], in0=iota16[:], scalar1=iota_cf[:, 0:1],
                scalar2=None, op0=IS_EQ,
            )

            # on-chip feature transforms: UwinT[loc,h], V8[lo,w,h] (fp16)
            for k, (lhs, rhs, dst) in enumerate(
                [(iw_s[:], w1i_s[:], UwinT[:])]
                + [
                    (txt16[:, w * P : (w + 1) * P], w1x_s[:], V8[:, w, :])
                    for w in range(NW)
                ]
            ):
                bp = psh.tile([P, HW + 8], F32, tag="h", name=f"bld{k}")
                nc.tensor.matmul(bp[:, 0:P], lhs, rhs, start=True, stop=True)
                nc.vector.tensor_copy(dst, bp[:, 0:P])

            acc = pso.tile([P, P], F32, tag="acc")
            for w in range(NW):
                e0 = w * bw
                # ---- phase A: h = relu(U-term + V-term + b1) ----
                h_ps = psh.tile([P, HW + 8], F32, tag="h")
                for o, n in ((0, 512), (512, bw - 512)):
                    nc.tensor.matmul(
                        h_ps[:, o : o + n], UwinT[:],
                        ohkt_s[:, e0 + o : e0 + o + n],
                        start=True, stop=False,
                    )
                    nc.tensor.matmul(
                        h_ps[:, o : o + n], V8[:, w, :],
                        ohlt_s[:, e0 + o : e0 + o + n],
                        start=False, stop=True,
                    )
                h16 = wp.tile([P, bw], F16, tag="h16")
                nc.scalar.activation(h16[:], h_ps[:, 0:bw], RELU, bias=b1c)
                # ---- za[e] = h_blk.T @ w2; a = sigmoid(za + b2) ----
                for j in range(bpw):
                    nc.tensor.matmul(
                        h_ps[:, HW + j : HW + j + 1],
                        h16[:, j * P : (j + 1) * P], w2h_s[:],
                        start=True, stop=True, skip_group_check=True,
                    )
                nc.scalar.activation(
                    a_s[:, w * bpw : (w + 1) * bpw],
                    h_ps[:, HW : HW + bpw], SIGMOID, bias=b2c,
                )
                # ---- phase B: ohka = ohk * a (one broadcast mult) ----
                ohkaB = wp.tile([P, bw], F16, tag="ohka")
                a_bc = a_s[:, w * bpw : (w + 1) * bpw, None].broadcast_to(
                    (P, bpw, P)
                )
                nc.vector.tensor_tensor(
                    out=ohkaB[:], in0=ohk_s[:, e0 : e0 + bw], in1=a_bc,
                    op=MULT,
                )
                # mtp bank layout (f32 cols): [0:128] M_T accum,
                # [128:256] part chunk, [256:320] M_N (f16 via bitcast)
                mtp = psm.tile([P, 384], F32, tag="mtp")
                for j in range(bpw):
                    sl = slice(e0 + j * P, e0 + (j + 1) * P)
                    nc.tensor.matmul(
                        mtp[:, 0:P], ohlo_s[:, sl], ohkaB[:, j * P : (j + 1) * P],
                        start=(j == 0), stop=(j == bpw - 1),
                        skip_group_check=True,
                    )
                m16T = wp.tile([P, P], F16, tag="m16T")
                nc.vector.tensor_copy(m16T[:], mtp[:, 0:P])
                mN_ps = mtp[:, 2 * P : 2 * P + P // 2].bitcast(F16)
                nc.tensor.matmul(
                    mN_ps, m16T[:], ident16[:], is_transpose=True,
                    start=True, stop=True, skip_group_check=True,
                )
                m16N = wp.tile([P, P], F16, tag="m16N")
                nc.vector.tensor_copy(m16N[:], mN_ps)
                # ---- tails ----
                nc.tensor.matmul(
                    acc[:], ttT8[:, w, :], m16T[:],
                    start=(w == 0), stop=(w == NW - 1), skip_group_check=True,
                )
                nc.tensor.matmul(
                    mtp[:, P : 2 * P], iwT_s[:], m16N[:],
                    start=True, stop=True, skip_group_check=True,
                )
                nc.vector.tensor_copy(
                    part_all[:, w * P : (w + 1) * P], mtp[:, P : 2 * P]
                )
                if w == NW // 2 - 1:
                    nc.sync.dma_start(
                        out_part[:, 0 : DIM // 2], part_all[:, 0 : DIM // 2]
                    )
                elif w == NW - 1:
                    nc.sync.dma_start(
                        out_part[:, DIM // 2 : DIM], part_all[:, DIM // 2 : DIM]
                    )

            out_sb = wp.tile([P, P], F32, tag="out_sb")
            nc.scalar.copy(out_sb[:], acc[:])
            nc.sync.dma_start(out_img[:], out_sb[:])

    nc.compile()
    return nc


_PROGRAMS = {}


def _get_program(bpw):
    if bpw not in _PROGRAMS:
        _PROGRAMS[bpw] = _build_program(bpw)
    return _PROGRAMS[bpw]


def _core_arrays(kloc, arb, bpw):
    """kloc: src-base (0..127) for this core's edges; arb: tgt values.
    Returns ohkt, ohlt [P, ec] (key-major) and ohk, ohlo [P, ec]
    (edge-major, per-block [e, key] tiles), all f16."""
    nblk = NW * bpw
    bw = bpw * P
    ec = nblk * P
    w = arb >> 7
    lo = arb & 127
    klocs = np.full(ec, -1, np.int64)
    los = np.full(ec, -1, np.int64)
    fill = np.zeros(NW, np.int64)
    order = np.argsort(w, kind="stable")
    for ei in order:
        wb = w[ei]
        s = wb * bw + fill[wb]
        klocs[s] = kloc[ei]
        los[s] = lo[ei]
        fill[wb] += 1
    import ml_dtypes

    f8 = ml_dtypes.float8_e4m3
    rng = np.arange(P)
    ohkt = np.ascontiguousarray((klocs[None, :] == rng[:, None]).astype(f8))
    ohlt = np.ascontiguousarray((los[None, :] == rng[:, None]).astype(f8))
    # edge-major: oh*[e % P, b*P + key] = (key_e == key)
    ohlo = np.zeros((P, ec), f8)
    ohk = np.zeros((P, ec), f8)
    losb = los.reshape(nblk, P)
    klocsb = klocs.reshape(nblk, P)
    for b in range(nblk):
        ohlo[:, b * P : (b + 1) * P] = (losb[b][:, None] == rng[None, :]).astype(f8)
        ohk[:, b * P : (b + 1) * P] = (klocsb[b][:, None] == rng[None, :]).astype(f8)
    return ohkt, ohlt, np.ascontiguousarray(ohk), np.ascontiguousarray(ohlo)


def _make_in_maps(img_features, text_features, src, tgt, W1, b1, w2, b2, bpw):
    img = np.asarray(img_features, dtype=np.float32)
    txt = np.asarray(text_features, dtype=np.float32)
    src = np.asarray(src).astype(np.int64)
    tgt = np.asarray(tgt).astype(np.int64)
    txt16 = np.ascontiguousarray(txt.astype(np.float16))
    txtT = txt.T.astype(np.float16)                     # [1024, 128]
    ttT8 = np.ascontiguousarray(
        txtT.reshape(NW, P, P).transpose(1, 0, 2).reshape(P, NW * P)
    )                                                   # [lo, w*128+b]
    w1i16 = W1[:, :P].T.astype(np.float16)
    w1x16 = W1[:, P:].T.astype(np.float16)
    cst = np.ascontiguousarray(
        np.stack(
            [np.asarray(b1, np.float32),
             np.full(P, np.float32(b2), np.float32)], axis=1)
    )
    w2h = np.asarray(w2, np.float16).reshape(P, 1)

    in_maps = []
    for c in range(NCORES):
        base = c * P
        sel = (src >= base) & (src < base + P)
        ohkt, ohlt, ohk, ohlo = _core_arrays(src[sel] - base, tgt[sel], bpw)
        iw = img[:, base : base + P].astype(np.float16)
        blob = np.ascontiguousarray(np.concatenate(
            [w1i16, w1x16, iw, np.ascontiguousarray(iw.T), w2h], axis=1))
        m = {
            "txt16": txt16, "ttT8": ttT8, "blob16": blob, "cst": cst,
            "ohkt": ohkt, "ohlt": ohlt, "ohlo": ohlo, "ohk": ohk,
        }
        in_maps.append(m)
    return in_maps


def _pick_bpw(src, tgt):
    src = np.asarray(src).astype(np.int64)
    tgt = np.asarray(tgt).astype(np.int64)
    mx = 0
    for c in range(NCORES):
        sel = (src >> 7) == c
        w = tgt[sel] >> 7
        mx = max(mx, int(np.bincount(w, minlength=NW).max()))
    return (mx + P - 1) // P


def _run(inputs, trace=False):
    from concourse.bass_utils import run_bass_kernel_spmd

    bpw = _pick_bpw(inputs["src"], inputs["tgt"])
    nc = _get_program(bpw)
    in_maps = _make_in_maps(**inputs, bpw=bpw)
    res = run_bass_kernel_spmd(
        nc, in_maps, core_ids=list(range(NCORES)), trace=trace
    )
    att_img = np.concatenate([r["out_img"] for r in res.results], axis=1)
    att_txt = np.sum([r["out_part"] for r in res.results], axis=0)
    return (
        np.ascontiguousarray(att_img.astype(np.float32)),
        np.ascontiguousarray(att_txt.astype(np.float32)),
    ), res


def kernel(**inputs):
    out, _ = _run(inputs, trace=False)
    return out


# revision 33
# speedup vs baseline: 1.4454x; 1.0082x over previous
"""Trainium2 Bass kernel for nn_CausalAttention (GNN message passing).

Math (reference):
    pairs[e] = [img[:, src[e]] ; text[:, tgt[e]]]          # B == H == 128
    a[e]     = sigmoid(w2 . relu(W1 @ pairs[e] + b1) + b2) # per-edge gate
    att_img[b, i] = sum_{e: src[e]=i} a[e] * text[b, tgt[e]]
    att_txt[b, t] = sum_{e: tgt[e]=t} a[e] * img[b, src[e]]

v5 architecture: deduplicated edges + host reduction, fp16 on-chip.
Core c owns the edges with src in Wc = [128c, 128c+128). It computes
att_img[:, Wc] exactly plus a PARTIAL att_txt[:, :]; the host sums the
8 partials (no collectives). Per-edge work is done once per edge.

Per core: edges bucketed by w = tgt>>7 (8 buckets, bpw 128-blocks each).
Host ships index-derived one-hot tables (f16):
  ohkt [loc, e] / ohlt [lo, e]  key-major (phase A gathers)
  ohk  [e, loc] / ohlo [e, lo]  edge-major (phase B outer products)
Per bucket w:
  h   = relu(UwinT.T @ ohkt + V8[w].T @ ohlt + b1)   4 matmuls + ACT
  za  = h_blk.T @ w2 (bpw N=1 matmuls), a = sigmoid(za + b2)
  ohka= ohk * a  (ONE broadcast tensor_tensor on DVE)
  M_T[lo, loc] += ohlo_blk.T @ ohka_blk  (bpw matmuls, PSUM group)
  M_N = PE-transpose(M_T) (1 matmul, f16 into bitcast PSUM region)
  acc_img += ttT8[w].T @ M_T             (1 matmul, long PSUM group)
  part[:, wc] = imgwinT.T @ M_N          (1 matmul) -> SBUF -> DMA out
PSUM banks: h x2 (4) + mtp x2 (2: mT | part | transposeT) + acc (1) = 7.
U/V feature transforms built on-chip in fp16 (9 matmuls).
"""

import sys

for _p in ("/opt/trn_rl_repo", "/root/.axon_site/_ro/trn_rl_repo"):
    if _p not in sys.path:
        sys.path.insert(0, _p)

import numpy as np

import concourse.bass as bass
import concourse.tile as tile
from concourse import bacc, mybir

P = 128
DIM = 1024
NCORES = 8
NW = 8            # tgt-hi buckets

F32 = mybir.dt.float32
F16 = mybir.dt.float16
F8 = mybir.dt.float8e4

IS_EQ = mybir.AluOpType.is_equal
MULT = mybir.AluOpType.mult
RELU = mybir.ActivationFunctionType.Relu
SIGMOID = mybir.ActivationFunctionType.Sigmoid


def _build_program(bpw):
    nblk = NW * bpw       # blocks total
    bw = bpw * P          # edge slots per bucket
    ec = nblk * P         # edge slots total

    nc = bacc.Bacc(None, target_bir_lowering=False, debug=False)

    txt16_d = nc.dram_tensor("txt16", [P, DIM], F16, kind="ExternalInput")
    ttT8_d = nc.dram_tensor("ttT8", [P, NW * P], F16, kind="ExternalInput")
    # blob: w1i | w1x | iw | iwT | w2h  (f16, 4*128+1 cols)
    blob_d = nc.dram_tensor("blob16", [P, 4 * P + 1], F16, kind="ExternalInput")
    cst_d = nc.dram_tensor("cst", [P, 2], F32, kind="ExternalInput")
    ohkt_d = nc.dram_tensor("ohkt", [P, ec], F8, kind="ExternalInput")
    ohlt_d = nc.dram_tensor("ohlt", [P, ec], F8, kind="ExternalInput")
    ohlo_d = nc.dram_tensor("ohlo", [P, ec], F8, kind="ExternalInput")
    ohk_d = nc.dram_tensor("ohk", [P, ec], F8, kind="ExternalInput")
    out_img = nc.dram_tensor("out_img", [P, P], F32, kind="ExternalOutput")
    out_part = nc.dram_tensor("out_part", [P, DIM], F32, kind="ExternalOutput")

    HW = 640  # h psum cols; za tail lives at cols HW:HW+bpw (same 2nd bank)

    with tile.TileContext(nc) as tc:
        with (
            tc.tile_pool(name="const", bufs=1) as cp,
            tc.tile_pool(name="work", bufs=4) as wp,
            tc.tile_pool(name="psh", bufs=2, space="PSUM") as psh,
            tc.tile_pool(name="psm", bufs=3, space="PSUM") as psm,
            tc.tile_pool(name="pso", bufs=1, space="PSUM") as pso,
        ):
            txt16 = cp.tile([P, DIM], F16)
            ttT8 = cp.tile([P, NW, P], F16)
            blob_s = cp.tile([P, 4 * P + 1], F16)
            cst_s = cp.tile([P, 2], F32)
            w1i_s = blob_s[:, 0:P]
            w1x_s = blob_s[:, P : 2 * P]
            iw_s = blob_s[:, 2 * P : 3 * P]
            iwT_s = blob_s[:, 3 * P : 4 * P]
            w2h_s = blob_s[:, 4 * P : 4 * P + 1]
            ohkt_s = cp.tile([P, ec], F8)
            ohlt_s = cp.tile([P, ec], F8)
            ohlo_s = cp.tile([P, ec], F8)
            ohk_s = cp.tile([P, ec], F8)
            part_all = cp.tile([P, DIM], F32)
            iota16 = cp.tile([P, P], F16)
            iota_i = cp.tile([P, 1], mybir.dt.int32)
            iota_cf = cp.tile([P, 1], F32)
            ident16 = cp.tile([P, P], F16)
            V8 = cp.tile([P, NW, P], F16)
            UwinT = cp.tile([P, P], F16)
            a_s = cp.tile([P, nblk], F32)

            # DMA plan: scalar(Act) queue issues only the big chunks of
            # ohlt/ohk; sync(SP) queue gets builds' inputs first, then
            # ohkt/ohlo chunks. Tables are fp8 (hold only 0/1). First
            # chunk covers buckets 0-1 so compute starts early.
            CH = [0, 2 * bw, 4 * bw, ec]  # chunks: b01 | b23 | b4567
            for a, b in zip(CH[:-1], CH[1:]):
                nc.scalar.dma_start(ohlt_s[:, a:b], ohlt_d[:, a:b])
                nc.scalar.dma_start(ohk_s[:, a:b], ohk_d[:, a:b])
            HD = DIM // 2
            nc.sync.dma_start(txt16[:, 0:HD], txt16_d[:, 0:HD])
            nc.sync.dma_start(blob_s[:], blob_d[:])
            nc.sync.dma_start(txt16[:, HD:DIM], txt16_d[:, HD:DIM])
            nc.sync.dma_start(cst_s[:], cst_d[:])
            for a, b in zip(CH[:-1], CH[1:]):
                nc.sync.dma_start(ohkt_s[:, a:b], ohkt_d[:, a:b])
                nc.sync.dma_start(ohlo_s[:, a:b], ohlo_d[:, a:b])
            nc.sync.dma_start(
                ttT8[:], ttT8_d[:].rearrange("p (w b) -> p w b", w=NW)
            )
            b1c = cst_s[:, 0:1]
            b2c = cst_s[:, 1:2]

            nc.gpsimd.iota(
                iota16[:], pattern=[[1, P]], base=0, channel_multiplier=0,
                allow_small_or_imprecise_dtypes=True,
            )
            nc.gpsimd.iota(iota_i[:], pattern=[[0, 1]], base=0,
                           channel_multiplier=1)
            nc.vector.tensor_copy(iota_cf[:], iota_i[:])
            nc.vector.tensor_scalar(
                out=ident16[:], in0=iota16[:], scalar1=iota_cf[:, 0:1],
                scalar2=None, op0=IS_EQ,
            )

            # on-chip feature transforms: UwinT[loc,h], V8[lo,w,h] (fp16)
            for k, (lhs, rhs, dst) in enumerate(
                [(iw_s[:], w1i_s[:], UwinT[:])]
                + [
                    (txt16[:, w * P : (w + 1) * P], w1x_s[:], V8[:, w, :])
                    for w in range(NW)
                ]
            ):
                bp = psh.tile([P, HW], F32, tag="h", name=f"bld{k}")
                nc.tensor.matmul(bp[:, 0:P], lhs, rhs, start=True, stop=True)
                nc.vector.tensor_copy(dst, bp[:, 0:P])

            acc = pso.tile([P, P], F32, tag="acc")
            for w in range(NW):
                e0 = w * bw
                # mtp bank layout (f32 cols): [0:128] M_T accum,
                # [128:256] part chunk, [256:320] M_N (f16 via bitcast),
                # [320:325] za.  All groups sequential within the bank.
                mtp = psm.tile([P, 384], F32, tag="mtp")
                # ---- phase A: h = relu(U-term + V-term + b1) ----
                h_ps = psh.tile([P, HW], F32, tag="h")
                for o, n in ((0, 512), (512, bw - 512)):
                    nc.tensor.matmul(
                        h_ps[:, o : o + n], UwinT[:],
                        ohkt_s[:, e0 + o : e0 + o + n],
                        start=True, stop=False,
                    )
                    nc.tensor.matmul(
                        h_ps[:, o : o + n], V8[:, w, :],
                        ohlt_s[:, e0 + o : e0 + o + n],
                        start=False, stop=True,
                    )
                h16 = wp.tile([P, bw], F16, tag="h16")
                nc.scalar.activation(h16[:], h_ps[:, 0:bw], RELU, bias=b1c)
                # ---- za[e] = h_blk.T @ w2; a = sigmoid(za + b2) ----
                for j in range(bpw):
                    nc.tensor.matmul(
                        mtp[:, 320 + j : 321 + j],
                        h16[:, j * P : (j + 1) * P], w2h_s[:],
                        start=True, stop=True, skip_group_check=True,
                    )
                nc.scalar.activation(
                    a_s[:, w * bpw : (w + 1) * bpw],
                    mtp[:, 320 : 320 + bpw], SIGMOID, bias=b2c,
                )
                # ---- phase B: ohka = ohk * a (one broadcast mult) ----
                ohkaB = wp.tile([P, bw], F16, tag="ohka")
                a_bc = a_s[:, w * bpw : (w + 1) * bpw, None].broadcast_to(
                    (P, bpw, P)
                )
                nc.vector.tensor_tensor(
                    out=ohkaB[:], in0=ohk_s[:, e0 : e0 + bw], in1=a_bc,
                    op=MULT,
                )
                for j in range(bpw):
                    sl = slice(e0 + j * P, e0 + (j + 1) * P)
                    nc.tensor.matmul(
                        mtp[:, 0:P], ohlo_s[:, sl], ohkaB[:, j * P : (j + 1) * P],
                        start=(j == 0), stop=(j == bpw - 1),
                        skip_group_check=True,
                    )
                m16T = wp.tile([P, P], F16, tag="m16T")
                nc.vector.tensor_copy(m16T[:], mtp[:, 0:P])
                mN_ps = mtp[:, 2 * P : 2 * P + P // 2].bitcast(F16)
                nc.tensor.matmul(
                    mN_ps, m16T[:], ident16[:], is_transpose=True,
                    start=True, stop=True, skip_group_check=True,
                )
                m16N = wp.tile([P, P], F16, tag="m16N")
                nc.vector.tensor_copy(m16N[:], mN_ps)
                # ---- tails ----
                nc.tensor.matmul(
                    acc[:], ttT8[:, w, :], m16T[:],
                    start=(w == 0), stop=(w == NW - 1), skip_group_check=True,
                )
                nc.tensor.matmul(
                    mtp[:, P : 2 * P], iwT_s[:], m16N[:],
                    start=True, stop=True, skip_group_check=True,
                )
                nc.vector.tensor_copy(
                    part_all[:, w * P : (w + 1) * P], mtp[:, P : 2 * P]
                )
                if w == NW // 2 - 1:
                    nc.sync.dma_start(
                        out_part[:, 0 : DIM // 2], part_all[:, 0 : DIM // 2]
                    )
                elif w == NW - 1:
                    nc.sync.dma_start(
                        out_part[:, DIM // 2 : DIM], part_all[:, DIM // 2 : DIM]
                    )

            out_sb = wp.tile([P, P], F32, tag="out_sb")
            nc.scalar.copy(out_sb[:], acc[:])
            nc.sync.dma_start(out_img[:], out_sb[:])

    nc.compile()
    return nc


_PROGRAMS = {}


def _get_program(bpw):
    if bpw not in _PROGRAMS:
        _PROGRAMS[bpw] = _build_program(bpw)
    return _PROGRAMS[bpw]


def _core_arrays(kloc, arb, bpw):
    """kloc: src-base (0..127) for this core's edges; arb: tgt values.
    Returns ohkt, ohlt [P, ec] (key-major) and ohk, ohlo [P, ec]
    (edge-major, per-block [e, key] tiles), all f16."""
    nblk = NW * bpw
    bw = bpw * P
    ec = nblk * P
    w = arb >> 7
    lo = arb & 127
    klocs = np.full(ec, -1, np.int64)
    los = np.full(ec, -1, np.int64)
    fill = np.zeros(NW, np.int64)
    order = np.argsort(w, kind="stable")
    for ei in order:
        wb = w[ei]
        s = wb * bw + fill[wb]
        klocs[s] = kloc[ei]
        los[s] = lo[ei]
        fill[wb] += 1
    import ml_dtypes

    f8 = ml_dtypes.float8_e4m3
    rng = np.arange(P)
    ohkt = np.ascontiguousarray((klocs[None, :] == rng[:, None]).astype(f8))
    ohlt = np.ascontiguousarray((los[None, :] == rng[:, None]).astype(f8))
    # edge-major: oh*[e % P, b*P + key] = (key_e == key)
    ohlo = np.zeros((P, ec), f8)
    ohk = np.zeros((P, ec), f8)
    losb = los.reshape(nblk, P)
    klocsb = klocs.reshape(nblk, P)
    for b in range(nblk):
        ohlo[:, b * P : (b + 1) * P] = (losb[b][:, None] == rng[None, :]).astype(f8)
        ohk[:, b * P : (b + 1) * P] = (klocsb[b][:, None] == rng[None, :]).astype(f8)
    return ohkt, ohlt, np.ascontiguousarray(ohk), np.ascontiguousarray(ohlo)


def _make_in_maps(img_features, text_features, src, tgt, W1, b1, w2, b2, bpw):
    img = np.asarray(img_features, dtype=np.float32)
    txt = np.asarray(text_features, dtype=np.float32)
    src = np.asarray(src).astype(np.int64)
    tgt = np.asarray(tgt).astype(np.int64)
    txt16 = np.ascontiguousarray(txt.astype(np.float16))
    txtT = txt.T.astype(np.float16)                     # [1024, 128]
    ttT8 = np.ascontiguousarray(
        txtT.reshape(NW, P, P).transpose(1, 0, 2).reshape(P, NW * P)
    )                                                   # [lo, w*128+b]
    w1i16 = W1[:, :P].T.astype(np.float16)
    w1x16 = W1[:, P:].T.astype(np.float16)
    cst = np.ascontiguousarray(
        np.stack(
            [np.asarray(b1, np.float32),
             np.full(P, np.float32(b2), np.float32)], axis=1)
    )
    w2h = np.asarray(w2, np.float16).reshape(P, 1)

    in_maps = []
    for c in range(NCORES):
        base = c * P
        sel = (src >= base) & (src < base + P)
        ohkt, ohlt, ohk, ohlo = _core_arrays(src[sel] - base, tgt[sel], bpw)
        iw = img[:, base : base + P].astype(np.float16)
        blob = np.ascontiguousarray(np.concatenate(
            [w1i16, w1x16, iw, np.ascontiguousarray(iw.T), w2h], axis=1))
        m = {
            "txt16": txt16, "ttT8": ttT8, "blob16": blob, "cst": cst,
            "ohkt": ohkt, "ohlt": ohlt, "ohlo": ohlo, "ohk": ohk,
        }
        in_maps.append(m)
    return in_maps


def _pick_bpw(src, tgt):
    src = np.asarray(src).astype(np.int64)
    tgt = np.asarray(tgt).astype(np.int64)
    mx = 0
    for c in range(NCORES):
        sel = (src >> 7) == c
        w = tgt[sel] >> 7
        mx = max(mx, int(np.bincount(w, minlength=NW).max()))
    return (mx + P - 1) // P


def _run(inputs, trace=False):
    from concourse.bass_utils import run_bass_kernel_spmd

    bpw = _pick_bpw(inputs["src"], inputs["tgt"])
    nc = _get_program(bpw)
    in_maps = _make_in_maps(**inputs, bpw=bpw)
    res = run_bass_kernel_spmd(
        nc, in_maps, core_ids=list(range(NCORES)), trace=trace
    )
    att_img = np.concatenate([r["out_img"] for r in res.results], axis=1)
    att_txt = np.sum([r["out_part"] for r in res.results], axis=0)
    return (
        np.ascontiguousarray(att_img.astype(np.float32)),
        np.ascontiguousarray(att_txt.astype(np.float32)),
    ), res


def kernel(**inputs):
    out, _ = _run(inputs, trace=False)
    return out


# revision 34
# speedup vs baseline: 1.4608x; 1.0107x over previous
"""Trainium2 Bass kernel for nn_CausalAttention (GNN message passing).

Math (reference):
    pairs[e] = [img[:, src[e]] ; text[:, tgt[e]]]          # B == H == 128
    a[e]     = sigmoid(w2 . relu(W1 @ pairs[e] + b1) + b2) # per-edge gate
    att_img[b, i] = sum_{e: src[e]=i} a[e] * text[b, tgt[e]]
    att_txt[b, t] = sum_{e: tgt[e]=t} a[e] * img[b, src[e]]

v9 architecture: deduplicated edges + host reduction, fp16/fp8 on-chip.
Core c owns the edges with src in Wc = [128c, 128c+128). It computes
att_img[:, Wc] exactly plus a PARTIAL att_txt[:, :]; the host sums the
8 partials (no collectives). Per-edge work is done once per edge.

Edges are bucketed by w = tgt>>7; each core PERMUTES its buckets by
fill (descending) so a shared capacity profile (max over cores of the
k-th largest fill, in 128-blocks) minimizes padding; the host permutes
txt/ttT8 chunks to match and un-permutes the out_part columns.

Host ships index-derived one-hot tables in fp8 (they hold only 0/1;
fp8 halves DMA, and the PE accepts f16 x f8 matmuls):
  ohkt [loc, e] / ohlt [lo, e]  key-major (phase A gathers)
  ohk  [e, loc] / ohlo [e, lo]  edge-major (phase B outer products)
Per bucket slot k (cap = 512 or 640 edge slots):
  h   = relu(UwinT.T @ ohkt + V8[k].T @ ohlt + b1)   2-4 mm + ACT
  za  = h_blk.T @ w2 (N=1 mms into mtp), a = sigmoid(za + b2)
  ohka= ohk * a  (ONE broadcast tensor_tensor on DVE)
  M_T[lo, loc] += ohlo_blk.T @ ohka_blk  (PSUM group in mtp)
  M_N = PE-transpose(M_T) (f16 into bitcast mtp region)
  acc_img += ttT8[k].T @ M_T             (long PSUM group)
  part[:, k] = imgwinT.T @ M_N           -> SBUF -> DMA out
PSUM banks: h x2 (4) + mtp x3 (3: mT|part|M_N|za) + acc (1) = 8.
U/V feature transforms built on-chip in fp16, interleaved with the
first buckets so the PE never queues behind un-arrived DMA chunks.
"""

import sys

for _p in ("/opt/trn_rl_repo", "/root/.axon_site/_ro/trn_rl_repo"):
    if _p not in sys.path:
        sys.path.insert(0, _p)

import numpy as np

import concourse.bass as bass
import concourse.tile as tile
from concourse import bacc, mybir

P = 128
DIM = 1024
NCORES = 8
NW = 8            # tgt-hi buckets

F32 = mybir.dt.float32
F16 = mybir.dt.float16
F8 = mybir.dt.float8e4

IS_EQ = mybir.AluOpType.is_equal
MULT = mybir.AluOpType.mult
RELU = mybir.ActivationFunctionType.Relu
SIGMOID = mybir.ActivationFunctionType.Sigmoid


def _build_program(blocks):
    caps = [b * P for b in blocks]           # slot capacities (edge slots)
    off = np.concatenate([[0], np.cumsum(caps)]).astype(int)
    ec = int(off[-1])

    nc = bacc.Bacc(None, target_bir_lowering=False, debug=False)

    txt16_d = nc.dram_tensor("txt16", [P, DIM], F16, kind="ExternalInput")
    ttT8_d = nc.dram_tensor("ttT8", [P, NW * P], F16, kind="ExternalInput")
    # blob: w1i | w1x | iw | iwT | w2h  (f16, 4*128+1 cols)
    blob_d = nc.dram_tensor("blob16", [P, 4 * P + 1], F16, kind="ExternalInput")
    cst_d = nc.dram_tensor("cst", [P, 2], F32, kind="ExternalInput")
    ohkt_d = nc.dram_tensor("ohkt", [P, ec], F8, kind="ExternalInput")
    ohlt_d = nc.dram_tensor("ohlt", [P, ec], F8, kind="ExternalInput")
    ohlo_d = nc.dram_tensor("ohlo", [P, ec], F8, kind="ExternalInput")
    ohk_d = nc.dram_tensor("ohk", [P, ec], F8, kind="ExternalInput")
    out_img = nc.dram_tensor("out_img", [P, P], F32, kind="ExternalOutput")
    out_part = nc.dram_tensor("out_part", [P, DIM], F32, kind="ExternalOutput")

    HW = 640

    with tile.TileContext(nc) as tc:
        with (
            tc.tile_pool(name="const", bufs=1) as cp,
            tc.tile_pool(name="work", bufs=4) as wp,
            tc.tile_pool(name="psh", bufs=2, space="PSUM") as psh,
            tc.tile_pool(name="psm", bufs=3, space="PSUM") as psm,
            tc.tile_pool(name="pso", bufs=1, space="PSUM") as pso,
        ):
            txt16 = cp.tile([P, DIM], F16)
            ttT8 = cp.tile([P, NW, P], F16)
            blob_s = cp.tile([P, 4 * P + 1], F16)
            cst_s = cp.tile([P, 2], F32)
            w1i_s = blob_s[:, 0:P]
            w1x_s = blob_s[:, P : 2 * P]
            iw_s = blob_s[:, 2 * P : 3 * P]
            iwT_s = blob_s[:, 3 * P : 4 * P]
            w2h_s = blob_s[:, 4 * P : 4 * P + 1]
            ohkt_s = cp.tile([P, ec], F8)
            ohlt_s = cp.tile([P, ec], F8)
            ohlo_s = cp.tile([P, ec], F8)
            ohk_s = cp.tile([P, ec], F8)
            part_all = cp.tile([P, DIM], F32)
            iota16 = cp.tile([P, P], F16)
            iota_i = cp.tile([P, 1], mybir.dt.int32)
            iota_cf = cp.tile([P, 1], F32)
            ident16 = cp.tile([P, P], F16)
            V8 = cp.tile([P, NW, P], F16)
            UwinT = cp.tile([P, P], F16)
            a_s = cp.tile([P, NW * 5], F32)

            # DMA plan: scalar(Act) queue issues the ohlt/ohk chunks;
            # sync(SP) gets builds' inputs first, then ohkt/ohlo chunks.
            # Chunks split at slot boundaries 2 and 4.
            CH = [0, int(off[2]), int(off[4]), ec]
            for a, b in zip(CH[:-1], CH[1:]):
                nc.scalar.dma_start(ohlt_s[:, a:b], ohlt_d[:, a:b])
                nc.scalar.dma_start(ohk_s[:, a:b], ohk_d[:, a:b])
            TC = [0, 2 * P, 4 * P, DIM]
            nc.sync.dma_start(txt16[:, TC[0] : TC[1]], txt16_d[:, TC[0] : TC[1]])
            nc.sync.dma_start(blob_s[:], blob_d[:])
            nc.sync.dma_start(txt16[:, TC[1] : TC[2]], txt16_d[:, TC[1] : TC[2]])
            nc.sync.dma_start(cst_s[:], cst_d[:])
            nc.sync.dma_start(ohkt_s[:, CH[0] : CH[1]], ohkt_d[:, CH[0] : CH[1]])
            nc.sync.dma_start(ohlo_s[:, CH[0] : CH[1]], ohlo_d[:, CH[0] : CH[1]])
            nc.sync.dma_start(txt16[:, TC[2] : TC[3]], txt16_d[:, TC[2] : TC[3]])
            nc.sync.dma_start(ohkt_s[:, CH[1] : CH[2]], ohkt_d[:, CH[1] : CH[2]])
            nc.sync.dma_start(ohlo_s[:, CH[1] : CH[2]], ohlo_d[:, CH[1] : CH[2]])
            nc.sync.dma_start(ohkt_s[:, CH[2] : CH[3]], ohkt_d[:, CH[2] : CH[3]])
            nc.sync.dma_start(ohlo_s[:, CH[2] : CH[3]], ohlo_d[:, CH[2] : CH[3]])
            nc.sync.dma_start(
                ttT8[:], ttT8_d[:].rearrange("p (w b) -> p w b", w=NW)
            )
            b1c = cst_s[:, 0:1]
            b2c = cst_s[:, 1:2]

            nc.gpsimd.iota(
                iota16[:], pattern=[[1, P]], base=0, channel_multiplier=0,
                allow_small_or_imprecise_dtypes=True,
            )
            nc.gpsimd.iota(iota_i[:], pattern=[[0, 1]], base=0,
                           channel_multiplier=1)
            nc.vector.tensor_copy(iota_cf[:], iota_i[:])
            nc.vector.tensor_scalar(
                out=ident16[:], in0=iota16[:], scalar1=iota_cf[:, 0:1],
                scalar2=None, op0=IS_EQ,
            )

            def build(lhs, rhs, dst, name):
                bp = psh.tile([P, HW], F32, tag="h", name=name)
                nc.tensor.matmul(bp[:, 0:P], lhs, rhs, start=True, stop=True)
                nc.vector.tensor_copy(dst, bp[:, 0:P])

            def vbuild(k):
                build(txt16[:, k * P : (k + 1) * P], w1x_s, V8[:, k, :],
                      f"v{k}")

            build(iw_s, w1i_s, UwinT[:], "u")
            vbuild(0)
            vbuild(1)

            acc = pso.tile([P, P], F32, tag="acc")
            for k in range(NW):
                cap = caps[k]
                nb = blocks[k]
                e0 = int(off[k])
                # interleave remaining V8 builds with the first buckets
                if k == 0:
                    vbuild(2), vbuild(3)
                elif k == 1:
                    vbuild(4), vbuild(5)
                elif k == 2:
                    vbuild(6), vbuild(7)
                # mtp bank layout (f32 cols): [0:128] M_T accum,
                # [128:256] part chunk, [256:320] M_N (f16 bitcast),
                # [320:325] za.  All groups sequential within the bank.
                mtp = psm.tile([P, 384], F32, tag="mtp")
                # ---- phase A: h = relu(U-term + V-term + b1) ----
                h_ps = psh.tile([P, HW], F32, tag="h")
                for o, n in ((0, 512), (512, cap - 512)):
                    if n <= 0:
                        continue
                    nc.tensor.matmul(
                        h_ps[:, o : o + n], UwinT[:],
                        ohkt_s[:, e0 + o : e0 + o + n],
                        start=True, stop=False,
                    )
                    nc.tensor.matmul(
                        h_ps[:, o : o + n], V8[:, k, :],
                        ohlt_s[:, e0 + o : e0 + o + n],
                        start=False, stop=True,
                    )
                h16 = wp.tile([P, HW], F16, tag="h16")
                nc.scalar.activation(
                    h16[:, 0:cap], h_ps[:, 0:cap], RELU, bias=b1c
                )
                # ---- za[e] = h_blk.T @ w2; a = sigmoid(za + b2) ----
                for j in range(nb):
                    nc.tensor.matmul(
                        mtp[:, 320 + j : 321 + j],
                        h16[:, j * P : (j + 1) * P], w2h_s,
                        start=True, stop=True, skip_group_check=True,
                    )
                nc.scalar.activation(
                    a_s[:, k * 5 : k * 5 + nb],
                    mtp[:, 320 : 320 + nb], SIGMOID, bias=b2c,
                )
                # ---- phase B: ohka = ohk * a (one broadcast mult) ----
                ohkaB = wp.tile([P, HW], F16, tag="ohka")
                a_bc = a_s[:, k * 5 : k * 5 + nb, None].broadcast_to(
                    (P, nb, P)
                )
                nc.vector.tensor_tensor(
                    out=ohkaB[:, 0:cap], in0=ohk_s[:, e0 : e0 + cap],
                    in1=a_bc, op=MULT,
                )
                for j in range(nb):
                    sl = slice(e0 + j * P, e0 + (j + 1) * P)
                    nc.tensor.matmul(
                        mtp[:, 0:P], ohlo_s[:, sl],
                        ohkaB[:, j * P : (j + 1) * P],
                        start=(j == 0), stop=(j == nb - 1),
                        skip_group_check=True,
                    )
                m16T = wp.tile([P, P], F16, tag="m16T")
                nc.vector.tensor_copy(m16T[:], mtp[:, 0:P])
                mN_ps = mtp[:, 2 * P : 2 * P + P // 2].bitcast(F16)
                nc.tensor.matmul(
                    mN_ps, m16T[:], ident16[:], is_transpose=True,
                    start=True, stop=True, skip_group_check=True,
                )
                m16N = wp.tile([P, P], F16, tag="m16N")
                nc.scalar.copy(m16N[:], mN_ps)
                # ---- tails ----
                nc.tensor.matmul(
                    acc[:], ttT8[:, k, :], m16T[:],
                    start=(k == 0), stop=(k == NW - 1), skip_group_check=True,
                )
                nc.tensor.matmul(
                    mtp[:, P : 2 * P], iwT_s, m16N[:],
                    start=True, stop=True, skip_group_check=True,
                )
                nc.vector.tensor_copy(
                    part_all[:, k * P : (k + 1) * P], mtp[:, P : 2 * P]
                )
                if k == NW // 2 - 1:
                    nc.sync.dma_start(
                        out_part[:, 0 : DIM // 2], part_all[:, 0 : DIM // 2]
                    )
                elif k == NW - 1:
                    nc.sync.dma_start(
                        out_part[:, DIM // 2 : DIM], part_all[:, DIM // 2 : DIM]
                    )

            out_sb = wp.tile([P, P], F32, tag="out_sb")
            nc.scalar.copy(out_sb[:], acc[:])
            nc.sync.dma_start(out_img[:], out_sb[:])

    nc.compile()
    return nc


_PROGRAMS = {}


def _get_program(blocks):
    key = tuple(blocks)
    if key not in _PROGRAMS:
        _PROGRAMS[key] = _build_program(list(blocks))
    return _PROGRAMS[key]


def _core_arrays(kloc, arb, order, blocks):
    """kloc: src-base (0..127) for this core's edges; arb: tgt values.
    order[k] = actual bucket handled by program slot k. Returns ohkt,
    ohlt (key-major), ohk, ohlo (edge-major) [P, ec] f8."""
    import ml_dtypes

    caps = [b * P for b in blocks]
    off = np.concatenate([[0], np.cumsum(caps)]).astype(int)
    ec = int(off[-1])
    w = arb >> 7
    lo = arb & 127
    klocs = np.full(ec, -1, np.int64)
    los = np.full(ec, -1, np.int64)
    slot_of = np.empty(NW, np.int64)
    slot_of[order] = np.arange(NW)
    fill = np.zeros(NW, np.int64)
    for i in range(len(kloc)):
        k = slot_of[w[i]]
        s = off[k] + fill[k]
        klocs[s] = kloc[i]
        los[s] = lo[i]
        fill[k] += 1
        assert fill[k] <= caps[k]
    f8 = ml_dtypes.float8_e4m3
    rng = np.arange(P)
    ohkt = np.ascontiguousarray((klocs[None, :] == rng[:, None]).astype(f8))
    ohlt = np.ascontiguousarray((los[None, :] == rng[:, None]).astype(f8))
    ohlo = np.zeros((P, ec), f8)
    ohk = np.zeros((P, ec), f8)
    nblk = ec // P
    losb = los.reshape(nblk, P)
    klocsb = klocs.reshape(nblk, P)
    for b in range(nblk):
        ohlo[:, b * P : (b + 1) * P] = (losb[b][:, None] == rng[None, :]).astype(f8)
        ohk[:, b * P : (b + 1) * P] = (klocsb[b][:, None] == rng[None, :]).astype(f8)
    return ohkt, ohlt, np.ascontiguousarray(ohk), np.ascontiguousarray(ohlo)


def _plan(src, tgt):
    """Per-core bucket order (fill desc) + shared capacity profile."""
    fills = np.zeros((NCORES, NW), np.int64)
    for c in range(NCORES):
        sel = (src >> 7) == c
        fills[c] = np.bincount(tgt[sel] >> 7, minlength=NW)
    orders = [np.argsort(-fills[c], kind="stable") for c in range(NCORES)]
    sorted_fills = -np.sort(-fills, axis=1)
    prof = sorted_fills.max(axis=0)
    blocks = [int(x) for x in np.ceil(prof / P).astype(int)]
    blocks = [min(max(b, 1), 5) for b in blocks]
    return orders, blocks


def _make_in_maps(img_features, text_features, src, tgt, W1, b1, w2, b2,
                  orders, blocks):
    img = np.asarray(img_features, dtype=np.float32)
    txt = np.asarray(text_features, dtype=np.float32)
    src = np.asarray(src).astype(np.int64)
    tgt = np.asarray(tgt).astype(np.int64)
    txt16f = txt.astype(np.float16)
    txtT = txt.T.astype(np.float16)                     # [1024, 128]
    w1i16 = W1[:, :P].T.astype(np.float16)
    w1x16 = W1[:, P:].T.astype(np.float16)
    cst = np.ascontiguousarray(
        np.stack(
            [np.asarray(b1, np.float32),
             np.full(P, np.float32(b2), np.float32)], axis=1)
    )
    w2h = np.asarray(w2, np.float16).reshape(P, 1)

    in_maps = []
    for c in range(NCORES):
        base = c * P
        order = orders[c]
        sel = (src >= base) & (src < base + P)
        ohkt, ohlt, ohk, ohlo = _core_arrays(
            src[sel] - base, tgt[sel], order, blocks
        )
        iw = img[:, base : base + P].astype(np.float16)
        blob = np.ascontiguousarray(np.concatenate(
            [w1i16, w1x16, iw, np.ascontiguousarray(iw.T), w2h], axis=1))
        # permute text chunks so program slot k sees bucket order[k]
        t16 = np.empty((P, DIM), np.float16)
        tt8 = np.empty((P, NW, P), np.float16)
        for k in range(NW):
            wv = order[k]
            t16[:, k * P : (k + 1) * P] = txt16f[:, wv * P : (wv + 1) * P]
            tt8[:, k, :] = txtT[wv * P : (wv + 1) * P, :]
        m = {
            "txt16": np.ascontiguousarray(t16),
            "ttT8": np.ascontiguousarray(tt8.reshape(P, NW * P)),
            "blob16": blob, "cst": cst,
            "ohkt": ohkt, "ohlt": ohlt, "ohlo": ohlo, "ohk": ohk,
        }
        in_maps.append(m)
    return in_maps


def _run(inputs, trace=False):
    from concourse.bass_utils import run_bass_kernel_spmd

    src = np.asarray(inputs["src"]).astype(np.int64)
    tgt = np.asarray(inputs["tgt"]).astype(np.int64)
    orders, blocks = _plan(src, tgt)
    nc = _get_program(blocks)
    in_maps = _make_in_maps(**inputs, orders=orders, blocks=blocks)
    res = run_bass_kernel_spmd(
        nc, in_maps, core_ids=list(range(NCORES)), trace=trace
    )
    att_img = np.concatenate([r["out_img"] for r in res.results], axis=1)
    att_txt = np.zeros((P, DIM), np.float32)
    for c in range(NCORES):
        part = res.results[c]["out_part"]
        order = orders[c]
        for k in range(NW):
            wv = order[k]
            att_txt[:, wv * P : (wv + 1) * P] += part[:, k * P : (k + 1) * P]
    return (
        np.ascontiguousarray(att_img.astype(np.float32)),
        np.ascontiguousarray(att_txt),
    ), res


def kernel(**inputs):
    out, _ = _run(inputs, trace=False)
    return out
